# revision 8
# baseline (speedup 1.0000x reference)
"""ChunkCrossAttention Trainium2 kernel (v2: fp8 + AllGather-KV).

Math (per reference):
  x = chunk_embeddings[0]                      # (S, L)
  k, v = split(x @ W_kv.T)                     # (S, D) each
  scores = einsum('jqd,sd->jqs', q, k) / sqrt(D), masked
  attn = softmax(scores, -1)
  out = (attn @ v) @ W_out.T + q  -> LayerNorm(gamma, beta)

Strategy (8 NeuronCores):
  - Queries sharded: each core owns 1024 q rows end-to-end (no partial
    softmax, no ReduceScatter of 8MB partials like v1).
  - W_out folded into the value projection on the host (Wvo = W_out @ Wv),
    so phase 1 emits k^T [d, s] and v' [s, d] directly in the layouts the
    attention matmuls need.
  - All matmuls fp8(e4m3) DoubleRow: 2x bf16 PE throughput. Weights are
    prescaled x64 on host (e4m3 min-normal 2^-6 vs W ~ N(0, 1/64^2));
    the psum->fp8 copies divide back by 64.
  - KV projection sharded over S (512 keys/core), then the tiny fp8 KV
    blob (264KB/core) is AllGather'd in 4 key-quarter chunks that
    pipeline behind phase-1/2 compute.
  - Softmax without max subtraction, with a global shift exp(x-3)
    (softmax-invariant) to keep fp8 exponents in range. exp runs on
    THREE engines in parallel: Act (native Exp -> fp8), DVE and GpSimd
    (Schraudolph: construct the e4m3 bit pattern directly as
    round(score*8*SCALE*log2e + const) with a saturating uint8 convert;
    negatives clamp to 0x00=+0.0, masked keys get bias -1e9 -> 0).
  - Denominator via a ones column appended to v' (DP=260 wide attn out).
  - LN scale invariance: LN(num/den + q) == LN(num + den*q), so the
    epilogue needs no reciprocal and no division.
  - Attention accumulates in PSUM across all 4096 keys: 6 q-tiles
    chunk-major (6 banks) + 2 q-tiles replayed at the end from staged
    exp tiles (frees 2 banks for the score matmuls during the main loop).
"""
import sys

sys.path.insert(0, "/opt/trn_rl_repo")

import numpy as np

import concourse.bacc as bacc
import concourse.mybir as mybir
import concourse.tile as tile
from concourse.bass_utils import run_bass_kernel_spmd

N_CORES = 8
J, Q, D = 64, 128, 256
S, L = 4096, 4096
S_LOC = S // N_CORES          # 512 keys per core
QR = (J // N_CORES) * Q       # 1024 query rows per core
DP = D + 4                    # attn free: 256 outputs + denom + 3 pad
LN_EPS = 1e-5
SCALE = 1.0 / np.sqrt(D)
SHIFT = 3.0                   # global exp shift (softmax invariant)
LOG2E = 1.4426950408889634
A_CONST = 8.0 * SCALE * LOG2E
BITS_B = 8.0 * (7.0 - SHIFT * LOG2E) - 0.5
WPRE = 64.0                   # host weight prescale before fp8 cast

F32 = mybir.dt.float32
FP8 = mybir.dt.float8e4
U8 = mybir.dt.uint8
AF = mybir.ActivationFunctionType
ALU = mybir.AluOpType
PM = mybir.MatmulPerfMode

N_LB = L // 256               # 16 L-pairs (256 contraction rows each)


def build_program():
    nc = bacc.Bacc(None, num_devices=N_CORES)

    xT8 = nc.declare_dram_parameter("xT8", [L, S_LOC], FP8, isOutput=False)
    wT8 = nc.declare_dram_parameter("wT8", [L, 2 * D], FP8, isOutput=False)
    qT8 = nc.declare_dram_parameter("qT8", [2 * 128, QR], FP8, isOutput=False)
    qres = nc.declare_dram_parameter("qres", [QR, D], F32, isOutput=False)
    dve_b = nc.declare_dram_parameter("dve_b", [128, 32], F32, isOutput=False)
    act_b = nc.declare_dram_parameter("act_b", [128, 32], F32, isOutput=False)
    gamma = nc.declare_dram_parameter("gamma", [D], F32, isOutput=False)
    beta = nc.declare_dram_parameter("beta", [D], F32, isOutput=False)
    y = nc.declare_dram_parameter("y", [QR, D], F32, isOutput=True)

    ag_in = [nc.dram_tensor(f"ag_in{qd}", [128, 516], FP8) for qd in range(4)]
    ag_out = [nc.dram_tensor(f"ag_out{qd}", [N_CORES, 128, 516], FP8,
                             addr_space="Shared") for qd in range(4)]

    import concourse.bass as bass

    with tile.TileContext(nc) as tc:
        with tc.tile_pool(name="singles", bufs=1) as singles, \
             tc.tile_pool(name="wpool", bufs=1) as wpool, \
             tc.tile_pool(name="xpool", bufs=N_LB + 4) as xpool, \
             tc.tile_pool(name="ex0", bufs=5) as ex0p, \
             tc.tile_pool(name="ex1", bufs=1) as ex1p, \
             tc.tile_pool(name="hp", bufs=6) as hpool, \
             tc.tile_pool(name="small", bufs=24) as small:

            # ---- persistent loads ----
            qT_sb = singles.tile([128, 2, QR], FP8)
            nc.gpsimd.dma_start(out=qT_sb,
                                in_=qT8.rearrange("(c p) q -> p c q", p=128))
            qres_sb = singles.tile([128, QR // 128, D], F32)
            nc.gpsimd.dma_start(out=qres_sb,
                                in_=qres.rearrange("(t p) d -> p t d", p=128))
            dve_b_sb = singles.tile([128, 32], F32)
            nc.gpsimd.dma_start(out=dve_b_sb, in_=dve_b[:, :])
            act_b_sb = singles.tile([128, 32], F32)
            nc.gpsimd.dma_start(out=act_b_sb, in_=act_b[:, :])
            g_ap = gamma[:]
            gamma_sb = singles.tile([128, D], F32)
            nc.gpsimd.dma_start(out=gamma_sb, in_=bass.AP(
                tensor=g_ap.tensor, offset=g_ap.offset,
                ap=[[0, 128], g_ap.ap[0]]))
            b_ap = beta[:]
            beta_sb = singles.tile([128, D], F32)
            nc.gpsimd.dma_start(out=beta_sb, in_=bass.AP(
                tensor=b_ap.tensor, offset=b_ap.offset,
                ap=[[0, 128], b_ap.ap[0]]))
            eps_sb = singles.tile([128, 1], F32)
            nc.vector.memset(eps_sb, LN_EPS)

            kT_loc = singles.tile([128, 2, S_LOC], FP8)
            vp_loc = singles.tile([128, 4, DP], FP8)
            nc.vector.memset(vp_loc[:, :, D:D + 1], 1.0)
            nc.vector.memset(vp_loc[:, :, D + 1:DP], 0.0)
            kT_all = singles.tile([128, N_CORES, 2, S_LOC], FP8)
            vp_all = singles.tile([128, N_CORES, 4, DP], FP8)

            # ---- phase 1: local KV projection (fp8 DR), AG per quarter ----
            ps1 = tc.tile_pool(name="ps1", bufs=1, space="PSUM")
            ps_1 = ps1.__enter__()
            kacc = [ps_1.tile([128, 2, 256], F32, name=f"kacc{h}")
                    for h in range(2)]
            vacc = [ps_1.tile([128, 256], F32, name=f"vacc{qd}")
                    for qd in range(4)]

            wts = []
            for lb in range(N_LB):
                wt = wpool.tile([128, 2, 2 * D], FP8, tag=f"wt{lb}",
                                name=f"wt{lb}")
                nc.sync.dma_start(
                    out=wt,
                    in_=wT8[lb * 256:(lb + 1) * 256, :].rearrange(
                        "(a p) n -> p a n", p=128))
                wts.append(wt)

            def kick_ag(qd):
                for dc in range(2):
                    nc.gpsimd.dma_start(
                        out=ag_in[qd][:, dc * 128:(dc + 1) * 128],
                        in_=kT_loc[:, dc, qd * 128:(qd + 1) * 128])
                nc.gpsimd.dma_start(out=ag_in[qd][:, 256:516],
                                    in_=vp_loc[:, qd, :])
                nc.gpsimd.collective_compute(
                    "AllGather", ALU.bypass,
                    replica_groups=[list(range(N_CORES))],
                    ins=[ag_in[qd][:, :]], outs=[ag_out[qd][:, :, :]])
                gathered = ag_out[qd].rearrange("c p f -> p c f")
                for dc in range(2):
                    nc.gpsimd.dma_start(
                        out=kT_all[:, :, dc, qd * 128:(qd + 1) * 128],
                        in_=gathered[:, :, dc * 128:(dc + 1) * 128])
                nc.gpsimd.dma_start(out=vp_all[:, :, qd, :],
                                    in_=gathered[:, :, 256:516])

            for h in range(2):
                xts = []
                for lb in range(N_LB):
                    xt = xpool.tile([128, 2, 256], FP8, tag="xt")
                    nc.sync.dma_start(
                        out=xt,
                        in_=xT8[lb * 256:(lb + 1) * 256,
                                h * 256:(h + 1) * 256].rearrange(
                                    "(a p) k -> p a k", p=128))
                    xts.append(xt)
                    for kd in range(2):
                        nc.tensor.matmul(
                            kacc[h][:, kd, :],
                            wts[lb][:, :, kd * 128:(kd + 1) * 128],
                            xt[:, :, :],
                            start=(lb == 0), stop=(lb == N_LB - 1),
                            perf_mode=PM.DoubleRow)
                nc.scalar.activation(out=kT_loc[:, :, h * 256:(h + 1) * 256],
                                     in_=kacc[h], func=AF.Copy,
                                     scale=1.0 / WPRE)
                for qq in range(2):
                    qd = 2 * h + qq
                    for lb in range(N_LB):
                        nc.tensor.matmul(
                            vacc[qd],
                            xts[lb][:, :, qq * 128:(qq + 1) * 128],
                            wts[lb][:, :, D:2 * D],
                            start=(lb == 0), stop=(lb == N_LB - 1),
                            perf_mode=PM.DoubleRow)
                    nc.scalar.activation(out=vp_loc[:, qd, 0:D], in_=vacc[qd],
                                         func=AF.Copy, scale=1.0 / WPRE)
                    kick_ag(qd)
            ps1.__exit__(None, None, None)

            # ---- phase 2: scores -> exp -> attention over all keys ----
            ps2a = tc.tile_pool(name="ps_at", bufs=1, space="PSUM")
            ps_at = ps2a.__enter__()
            ps2s = tc.tile_pool(name="ps_sc", bufs=2, space="PSUM")
            ps_sc = ps2s.__enter__()

            at = [ps_at.tile([128, DP], F32, name=f"at{qt}") for qt in range(6)]
            ex1_tiles = {}
            for b in range(N_CORES):
                for pc in range(2):
                    ex1_tiles[(b, pc)] = ex1p.tile(
                        [128, 2, 512], FP8, name=f"ex1_{b}_{pc}")

            # exp engine schedule: alternate Act / DVE (gpsimd cannot
            # read PSUM, so it gets the SBUF-side epilogue ops instead)
            def do_exp(ext_slice, sc, key_idx, tag):
                if tag % 2 == 0:
                    nc.scalar.activation(out=ext_slice, in_=sc, func=AF.Exp,
                                         bias=act_b_sb[:, key_idx:key_idx + 1],
                                         scale=SCALE)
                else:
                    nc.vector.tensor_scalar(out=ext_slice.bitcast(U8), in0=sc,
                                            scalar1=A_CONST,
                                            scalar2=dve_b_sb[:, key_idx:key_idx + 1],
                                            op0=ALU.mult, op1=ALU.add)

            tag = 0
            for pc in range(2):
                for b in range(N_CORES):
                    ex0t = ex0p.tile([128, 2, 512], FP8, tag="ex0",
                                     name="ex0t")
                    ex_pair = [ex0t, ex1_tiles[(b, pc)]]
                    for sub in range(2):
                        st = 2 * pc + sub
                        for qh in range(2):
                            sc = ps_sc.tile([128, 512], F32, tag="sc")
                            nc.tensor.matmul(
                                sc,
                                kT_all[:, b, :, st * 128:(st + 1) * 128],
                                qT_sb[:, :, qh * 512:(qh + 1) * 512],
                                start=True, stop=True,
                                perf_mode=PM.DoubleRow)
                            do_exp(ex_pair[qh][:, sub, :], sc, 4 * b + st, tag)
                            tag += 1
                    for qt in range(6):
                        qh, qi = (0, qt) if qt < 4 else (1, qt - 4)
                        nc.tensor.matmul(
                            at[qt], ex_pair[qh][:, :, qi * 128:(qi + 1) * 128],
                            vp_all[:, b, 2 * pc:2 * pc + 2, :],
                            start=(pc == 0 and b == 0),
                            stop=(pc == 1 and b == N_CORES - 1),
                            perf_mode=PM.DoubleRow)

            ps2s.__exit__(None, None, None)

            # deferred q-tiles 6,7 replay staged exp against vp
            ps2d = tc.tile_pool(name="ps_d", bufs=1, space="PSUM")
            ps_d = ps2d.__enter__()
            at_d = [ps_d.tile([128, DP], F32, name=f"atd{i}") for i in range(2)]
            for i, qt in enumerate((6, 7)):
                first = True
                for pc in range(2):
                    for b in range(N_CORES):
                        nc.tensor.matmul(
                            at_d[i],
                            ex1_tiles[(b, pc)][:, :, (qt - 4) * 128:(qt - 3) * 128],
                            vp_all[:, b, 2 * pc:2 * pc + 2, :],
                            start=first, stop=(pc == 1 and b == N_CORES - 1),
                            perf_mode=PM.DoubleRow)
                        first = False

            # ---- epilogue: h' = num + den*q, LN (scale-invariant) ----
            y_r = y.rearrange("(t p) d -> t p d", p=128)

            def epilogue(qt, acc):
                den = small.tile([128, 1], F32, tag="den")
                nc.vector.tensor_copy(out=den, in_=acc[:, D:D + 1])
                h = hpool.tile([128, D], F32, tag="h")
                nc.vector.tensor_scalar_mul(out=h, in0=qres_sb[:, qt, :],
                                            scalar1=den)
                nc.vector.tensor_add(out=h, in0=h, in1=acc[:, 0:D])
                stats = small.tile([128, 6], F32, tag="stats")
                nc.vector.bn_stats(out=stats, in_=h)
                mv = small.tile([128, 2], F32, tag="mv")
                nc.vector.bn_aggr(out=mv, in_=stats)
                rstd = small.tile([128, 1], F32, tag="rstd")
                nc.scalar.activation(out=rstd, in_=mv[:, 1:2], func=AF.Sqrt,
                                     bias=eps_sb, scale=1.0)
                nc.vector.reciprocal(out=rstd, in_=rstd)
                nmr = small.tile([128, 1], F32, tag="nmr")
                nc.vector.tensor_scalar(out=nmr, in0=mv[:, 0:1],
                                        scalar1=rstd, scalar2=-1.0,
                                        op0=ALU.mult, op1=ALU.mult)
                xh = hpool.tile([128, D], F32, tag="xh")
                nc.scalar.activation(out=xh, in_=h, func=AF.Identity,
                                     bias=nmr, scale=rstd)
                nc.gpsimd.tensor_mul(out=xh, in0=xh, in1=gamma_sb)
                nc.gpsimd.tensor_add(out=xh, in0=xh, in1=beta_sb)
                nc.sync.dma_start(out=y_r[qt], in_=xh)

            for qt in range(6):
                epilogue(qt, at[qt])
            for i, qt in enumerate((6, 7)):
                epilogue(qt, at_d[i])

            ps2d.__exit__(None, None, None)
            ps2a.__exit__(None, None, None)

    nc.finalize()
    return nc


_NC_CACHE = None


def _make_in_maps(inputs):
    import ml_dtypes
    e4 = ml_dtypes.float8_e4m3

    def q8(a):
        return np.clip(a, -240.0, 240.0).astype(e4)

    jq = np.asarray(inputs["justice_queries"], dtype=np.float32)
    x = np.asarray(inputs["chunk_embeddings"], dtype=np.float32)[0]
    mask = np.asarray(inputs["chunk_mask"])
    wkv = np.asarray(inputs["W_kv"], dtype=np.float32)
    wout = np.asarray(inputs["W_out"], dtype=np.float32)
    gamma = np.asarray(inputs["ln_gamma"], dtype=np.float32)
    beta = np.asarray(inputs["ln_beta"], dtype=np.float32)

    wk = wkv[:D]
    wvo = wout @ wkv[D:]                          # fold W_out into Wv
    wT = np.concatenate([wk, wvo], axis=0) * WPRE  # (512, L)
    wT8 = np.ascontiguousarray(q8(wT.T))           # (L, 512)
    xT8_full = np.ascontiguousarray(q8(x.T))       # (L, S)

    flat = np.ascontiguousarray(jq.reshape(J * Q, D))
    mask_on = mask != 0
    # per-key exp biases, laid out [p, b*4+st] for key = b*512+st*128+p
    dve_b = np.empty((128, 32), dtype=np.float32)
    act_b = np.empty((128, 32), dtype=np.float32)
    for col in range(32):
        b_, st = col // 4, col % 4
        keys = b_ * 512 + st * 128 + np.arange(128)
        on = mask_on[keys]
        dve_b[:, col] = np.where(on, BITS_B, -1e9)
        act_b[:, col] = np.where(on, -SHIFT, -1e30)

    in_maps = []
    for c in range(N_CORES):
        qrows = flat[c * QR:(c + 1) * QR]          # (1024, 256)
        qT = np.ascontiguousarray(qrows.T)         # (256, 1024)
        in_maps.append({
            "xT8": np.ascontiguousarray(
                xT8_full[:, c * S_LOC:(c + 1) * S_LOC]),
            "wT8": wT8,
            "qT8": np.ascontiguousarray(q8(qT)),
            "qres": np.ascontiguousarray(qrows),
            "dve_b": dve_b,
            "act_b": act_b,
            "gamma": gamma,
            "beta": beta,
        })
    return in_maps


def kernel(**inputs) -> np.ndarray:
    global _NC_CACHE
    in_maps = _make_in_maps(inputs)
    if _NC_CACHE is None:
        _NC_CACHE = build_program()
    res = run_bass_kernel_spmd(_NC_CACHE, in_maps, list(range(N_CORES)))
    out = np.concatenate([res.results[c]["y"] for c in range(N_CORES)], axis=0)
    return np.ascontiguousarray(out.reshape(J, Q, D).astype(np.float32))


# revision 14
# speedup vs baseline: 1.2249x; 1.2249x over previous
"""ChunkCrossAttention Trainium2 kernel (v2: fp8 + AllGather-KV).

Math (per reference):
  x = chunk_embeddings[0]                      # (S, L)
  k, v = split(x @ W_kv.T)                     # (S, D) each
  scores = einsum('jqd,sd->jqs', q, k) / sqrt(D), masked
  attn = softmax(scores, -1)
  out = (attn @ v) @ W_out.T + q  -> LayerNorm(gamma, beta)

Strategy (8 NeuronCores):
  - Queries sharded: each core owns 1024 q rows end-to-end (no partial
    softmax, no ReduceScatter of 8MB partials like v1).
  - W_out folded into the value projection on the host (Wvo = W_out @ Wv),
    so phase 1 emits k^T [d, s] and v' [s, d] directly in the layouts the
    attention matmuls need.
  - All matmuls fp8(e4m3) DoubleRow: 2x bf16 PE throughput. Weights are
    prescaled x64 on host (e4m3 min-normal 2^-6 vs W ~ N(0, 1/64^2));
    the psum->fp8 copies divide back by 64.
  - KV projection sharded over S (512 keys/core), then the tiny fp8 KV
    blob (264KB/core) is AllGather'd in 4 key-quarter chunks that
    pipeline behind phase-1/2 compute.
  - Softmax without max subtraction, with a global shift exp(x-3)
    (softmax-invariant) to keep fp8 exponents in range. exp runs on
    THREE engines in parallel: Act (native Exp -> fp8), DVE and GpSimd
    (Schraudolph: construct the e4m3 bit pattern directly as
    round(score*8*SCALE*log2e + const) with a saturating uint8 convert;
    negatives clamp to 0x00=+0.0, masked keys get bias -1e9 -> 0).
  - Denominator via a ones column appended to v' (DP=260 wide attn out).
  - LN scale invariance: LN(num/den + q) == LN(num + den*q), so the
    epilogue needs no reciprocal and no division.
  - Attention accumulates in PSUM across all 4096 keys: 6 q-tiles
    chunk-major (6 banks) + 2 q-tiles replayed at the end from staged
    exp tiles (frees 2 banks for the score matmuls during the main loop).
"""
import sys

sys.path.insert(0, "/opt/trn_rl_repo")

import numpy as np

import concourse.bacc as bacc
import concourse.mybir as mybir
import concourse.tile as tile
from concourse.bass_utils import run_bass_kernel_spmd

N_CORES = 8
J, Q, D = 64, 128, 256
S, L = 4096, 4096
S_LOC = S // N_CORES          # 512 keys per core
QR = (J // N_CORES) * Q       # 1024 query rows per core
DP = D + 4                    # attn free: 256 outputs + denom + 3 pad
LN_EPS = 1e-5
SCALE = 1.0 / np.sqrt(D)
SHIFT = 3.0                   # global exp shift (softmax invariant)
LOG2E = 1.4426950408889634
A_CONST = 8.0 * SCALE * LOG2E
BITS_B = 8.0 * (7.0 - SHIFT * LOG2E) - 0.5
WPRE = 64.0                   # host weight prescale before fp8 cast

F32 = mybir.dt.float32
FP8 = mybir.dt.float8e4
U8 = mybir.dt.uint8
AF = mybir.ActivationFunctionType
ALU = mybir.AluOpType
PM = mybir.MatmulPerfMode

N_LB = L // 256               # 16 L-pairs (256 contraction rows each)


def build_program():
    nc = bacc.Bacc(None, num_devices=N_CORES)

    xT8 = nc.declare_dram_parameter("xT8", [L, S_LOC], FP8, isOutput=False)
    wT8 = nc.declare_dram_parameter("wT8", [L, 2 * D], FP8, isOutput=False)
    qT8 = nc.declare_dram_parameter("qT8", [2 * 128, QR], FP8, isOutput=False)
    qres = nc.declare_dram_parameter("qres", [QR, D], F32, isOutput=False)
    dve_b = nc.declare_dram_parameter("dve_b", [128, 32], F32, isOutput=False)
    act_b = nc.declare_dram_parameter("act_b", [128, 32], F32, isOutput=False)
    gamma = nc.declare_dram_parameter("gamma", [D], F32, isOutput=False)
    beta = nc.declare_dram_parameter("beta", [D], F32, isOutput=False)
    y = nc.declare_dram_parameter("y", [QR, D], F32, isOutput=True)

    ag_in = [nc.dram_tensor(f"ag_in{h}", [128, 2, 516], FP8) for h in range(2)]
    ag_out = [nc.dram_tensor(f"ag_out{h}", [N_CORES, 128, 2, 516], FP8,
                             addr_space="Shared") for h in range(2)]
    ag_in_d = nc.dram_tensor("ag_in_d", [128, 4], FP8)
    ag_out_d = nc.dram_tensor("ag_out_d", [N_CORES, 128, 4], FP8,
                              addr_space="Shared")

    import concourse.bass as bass

    with tile.TileContext(nc) as tc:
        with tc.tile_pool(name="singles", bufs=1) as singles, \
             tc.tile_pool(name="wpool", bufs=1) as wpool, \
             tc.tile_pool(name="xpool", bufs=N_LB + 4) as xpool, \
             tc.tile_pool(name="ex0", bufs=5) as ex0p, \
             tc.tile_pool(name="ex1", bufs=1) as ex1p, \
             tc.tile_pool(name="hp", bufs=6) as hpool, \
             tc.tile_pool(name="small", bufs=24) as small:

            # ---- dummy collective: absorbs cross-core start skew and cc
            # stream init while phase 1 computes, so the real AllGathers
            # pay only their wire time ----
            dummy_sb = singles.tile([128, 4], FP8)
            nc.gpsimd.memset(dummy_sb, 0.0)
            nc.gpsimd.dma_start(out=ag_in_d[:, :], in_=dummy_sb)
            nc.gpsimd.collective_compute(
                "AllGather", ALU.bypass,
                replica_groups=[list(range(N_CORES))],
                ins=[ag_in_d[:, :]], outs=[ag_out_d[:, :, :]])

            # ---- persistent loads (scalar DMA queue; sync/vector carry
            # the phase-1 streams) ----
            qT_sb = singles.tile([128, 2, QR], FP8)
            nc.gpsimd.dma_start(out=qT_sb,
                                in_=qT8.rearrange("(c p) q -> p c q", p=128))
            dve_b_sb = singles.tile([128, 32], F32)
            nc.gpsimd.dma_start(out=dve_b_sb, in_=dve_b[:, :])
            act_b_sb = singles.tile([128, 32], F32)
            nc.gpsimd.dma_start(out=act_b_sb, in_=act_b[:, :])
            qres_sb = singles.tile([128, QR // 128, D], F32)
            for qq in range(2):
                nc.gpsimd.dma_start(
                    out=qres_sb[:, qq * 4:(qq + 1) * 4, :],
                    in_=qres.rearrange("(t p) d -> p t d", p=128)[
                        :, qq * 4:(qq + 1) * 4, :])
            g_ap = gamma[:]
            gamma_sb = singles.tile([128, D], F32)
            nc.gpsimd.dma_start(out=gamma_sb, in_=bass.AP(
                tensor=g_ap.tensor, offset=g_ap.offset,
                ap=[[0, 128], g_ap.ap[0]]))
            b_ap = beta[:]
            beta_sb = singles.tile([128, D], F32)
            nc.gpsimd.dma_start(out=beta_sb, in_=bass.AP(
                tensor=b_ap.tensor, offset=b_ap.offset,
                ap=[[0, 128], b_ap.ap[0]]))
            eps_sb = singles.tile([128, 1], F32)
            nc.vector.memset(eps_sb, LN_EPS)

            kT_loc = singles.tile([128, 2, S_LOC], FP8)
            vp_loc = singles.tile([128, 4, DP], FP8)
            nc.vector.memset(vp_loc[:, :, D:D + 1], 1.0)
            nc.vector.memset(vp_loc[:, :, D + 1:DP], 0.0)
            kT_all = singles.tile([128, N_CORES, 2, S_LOC], FP8)
            vp_all = singles.tile([128, N_CORES, 4, DP], FP8)

            # ---- phase 1: local KV projection (fp8 DR), AG per quarter ----
            ps1 = tc.tile_pool(name="ps1", bufs=1, space="PSUM")
            ps_1 = ps1.__enter__()
            kacc = [ps_1.tile([128, 2, 256], F32, name=f"kacc{h}")
                    for h in range(2)]
            vacc = [ps_1.tile([128, 256], F32, name=f"vacc{qd}")
                    for qd in range(4)]

            wts = []
            for lb in range(N_LB):
                wt = wpool.tile([128, 2, 2 * D], FP8, tag=f"wt{lb}",
                                name=f"wt{lb}")
                nc.sync.dma_start(
                    out=wt,
                    in_=wT8[lb * 256:(lb + 1) * 256, :].rearrange(
                        "(a p) n -> p a n", p=128))
                wts.append(wt)

            def kick_ag(h):
                # blob [128, 2, 516]: [:, dc, 0:256]=kT dc-chunk,
                # [:, j, 256:516]=vp quarter 2h+j
                for dc in range(2):
                    nc.gpsimd.dma_start(
                        out=ag_in[h][:, dc, 0:256],
                        in_=kT_loc[:, dc, h * 256:(h + 1) * 256])
                for j in range(2):
                    nc.gpsimd.dma_start(out=ag_in[h][:, j, 256:516],
                                        in_=vp_loc[:, 2 * h + j, :])
                nc.gpsimd.collective_compute(
                    "AllGather", ALU.bypass,
                    replica_groups=[list(range(N_CORES))],
                    ins=[ag_in[h][:, :, :]], outs=[ag_out[h][:, :, :, :]])
                gathered = ag_out[h].rearrange("c p g f -> p c g f")
                for dc in range(2):
                    nc.gpsimd.dma_start(
                        out=kT_all[:, :, dc, h * 256:(h + 1) * 256],
                        in_=gathered[:, :, dc, 0:256])
                for j in range(2):
                    nc.gpsimd.dma_start(out=vp_all[:, :, 2 * h + j, :],
                                        in_=gathered[:, :, j, 256:516])

            for h in range(2):
                xts = []
                for xc in range(N_LB // 2):
                    xt = xpool.tile([128, 2, 2, 256], FP8, tag="xt")
                    nc.scalar.dma_start(
                        out=xt,
                        in_=xT8[xc * 512:(xc + 1) * 512,
                                h * 256:(h + 1) * 256].rearrange(
                                    "(l a p) k -> p l a k", p=128, a=2))
                    xts.append(xt)
                for lb in range(N_LB):
                    xt_s = xts[lb // 2][:, lb % 2, :, :]
                    for kd in range(2):
                        nc.tensor.matmul(
                            kacc[h][:, kd, :],
                            wts[lb][:, :, kd * 128:(kd + 1) * 128],
                            xt_s,
                            start=(lb == 0), stop=(lb == N_LB - 1),
                            perf_mode=PM.DoubleRow)
                nc.scalar.activation(out=kT_loc[:, :, h * 256:(h + 1) * 256],
                                     in_=kacc[h], func=AF.Copy,
                                     scale=1.0 / WPRE)
                for qq in range(2):
                    qd = 2 * h + qq
                    for lb in range(N_LB):
                        nc.tensor.matmul(
                            vacc[qd],
                            xts[lb // 2][:, lb % 2, :,
                                         qq * 128:(qq + 1) * 128],
                            wts[lb][:, :, D:2 * D],
                            start=(lb == 0), stop=(lb == N_LB - 1),
                            perf_mode=PM.DoubleRow)
                    nc.scalar.activation(out=vp_loc[:, qd, 0:D], in_=vacc[qd],
                                         func=AF.Copy, scale=1.0 / WPRE)
                kick_ag(h)
            ps1.__exit__(None, None, None)

            # ---- phase 2: scores -> exp -> attention over all keys ----
            ps2a = tc.tile_pool(name="ps_at", bufs=1, space="PSUM")
            ps_at = ps2a.__enter__()
            ps2s = tc.tile_pool(name="ps_sc", bufs=2, space="PSUM")
            ps_sc = ps2s.__enter__()

            at = [ps_at.tile([128, DP], F32, name=f"at{qt}") for qt in range(6)]
            ex1_tiles = {}
            for b in range(N_CORES):
                for pc in range(2):
                    ex1_tiles[(b, pc)] = ex1p.tile(
                        [128, 2, 512], FP8, name=f"ex1_{b}_{pc}")

            # exp engine split: alternate Act / DVE per key-subtile so
            # every q row averages both engines' quantization error.
            # (gpsimd cannot read PSUM, so it only gets SBUF-side work.)
            def do_exp(ext_slice, sc, b, st, qh):
                key_idx = 4 * b + st
                if (b + st + qh) % 2 == 0:
                    nc.scalar.activation(out=ext_slice, in_=sc, func=AF.Exp,
                                         bias=act_b_sb[:, key_idx:key_idx + 1],
                                         scale=SCALE)
                else:
                    nc.vector.tensor_scalar(out=ext_slice.bitcast(U8), in0=sc,
                                            scalar1=A_CONST,
                                            scalar2=dve_b_sb[:, key_idx:key_idx + 1],
                                            op0=ALU.mult, op1=ALU.add)

            for pc in range(2):
                for b in range(N_CORES):
                    ex0t = ex0p.tile([128, 2, 512], FP8, tag="ex0",
                                     name="ex0t")
                    ex_pair = [ex0t, ex1_tiles[(b, pc)]]
                    for qh in range(2):
                        for sub in range(2):
                            st = 2 * pc + sub
                            sc = ps_sc.tile([128, 512], F32, tag="sc")
                            nc.tensor.matmul(
                                sc,
                                kT_all[:, b, :, st * 128:(st + 1) * 128],
                                qT_sb[:, :, qh * 512:(qh + 1) * 512],
                                start=True, stop=True,
                                perf_mode=PM.DoubleRow)
                            do_exp(ex_pair[qh][:, sub, :], sc, b, st, qh)
                    for qt in range(6):
                        qh, qi = (0, qt) if qt < 4 else (1, qt - 4)
                        nc.tensor.matmul(
                            at[qt], ex_pair[qh][:, :, qi * 128:(qi + 1) * 128],
                            vp_all[:, b, 2 * pc:2 * pc + 2, :],
                            start=(pc == 0 and b == 0),
                            stop=(pc == 1 and b == N_CORES - 1),
                            perf_mode=PM.DoubleRow)

            ps2s.__exit__(None, None, None)

            # deferred q-tiles 6,7 replay staged exp against vp
            ps2d = tc.tile_pool(name="ps_d", bufs=1, space="PSUM")
            ps_d = ps2d.__enter__()
            at_d = [ps_d.tile([128, DP], F32, name=f"atd{i}") for i in range(2)]
            for i, qt in enumerate((6, 7)):
                first = True
                for pc in range(2):
                    for b in range(N_CORES):
                        nc.tensor.matmul(
                            at_d[i],
                            ex1_tiles[(b, pc)][:, :, (qt - 4) * 128:(qt - 3) * 128],
                            vp_all[:, b, 2 * pc:2 * pc + 2, :],
                            start=first, stop=(pc == 1 and b == N_CORES - 1),
                            perf_mode=PM.DoubleRow)
                        first = False

            # ---- epilogue: h' = num + den*q, LN (scale-invariant) ----
            y_r = y.rearrange("(t p) d -> t p d", p=128)

            def epilogue(qt, acc):
                den = small.tile([128, 1], F32, tag="den")
                nc.vector.tensor_copy(out=den, in_=acc[:, D:D + 1])
                h = hpool.tile([128, D], F32, tag="h")
                nc.vector.tensor_scalar_mul(out=h, in0=qres_sb[:, qt, :],
                                            scalar1=den)
                nc.vector.tensor_add(out=h, in0=h, in1=acc[:, 0:D])
                stats = small.tile([128, 6], F32, tag="stats")
                nc.vector.bn_stats(out=stats, in_=h)
                mv = small.tile([128, 2], F32, tag="mv")
                nc.vector.bn_aggr(out=mv, in_=stats)
                rstd = small.tile([128, 1], F32, tag="rstd")
                nc.scalar.activation(out=rstd, in_=mv[:, 1:2], func=AF.Sqrt,
                                     bias=eps_sb, scale=1.0)
                nc.vector.reciprocal(out=rstd, in_=rstd)
                nmr = small.tile([128, 1], F32, tag="nmr")
                nc.vector.tensor_scalar(out=nmr, in0=mv[:, 0:1],
                                        scalar1=rstd, scalar2=-1.0,
                                        op0=ALU.mult, op1=ALU.mult)
                xh = hpool.tile([128, D], F32, tag="xh")
                nc.scalar.activation(out=xh, in_=h, func=AF.Identity,
                                     bias=nmr, scale=rstd)
                nc.vector.tensor_mul(out=xh, in0=xh, in1=gamma_sb)
                nc.vector.tensor_add(out=xh, in0=xh, in1=beta_sb)
                nc.sync.dma_start(out=y_r[qt], in_=xh)

            for qt in range(6):
                epilogue(qt, at[qt])
            for i, qt in enumerate((6, 7)):
                epilogue(qt, at_d[i])

            ps2d.__exit__(None, None, None)
            ps2a.__exit__(None, None, None)

    nc.finalize()
    return nc


_NC_CACHE = None


def _make_in_maps(inputs):
    import ml_dtypes
    e4 = ml_dtypes.float8_e4m3

    def q8(a):
        return np.clip(a, -240.0, 240.0).astype(e4)

    jq = np.asarray(inputs["justice_queries"], dtype=np.float32)
    x = np.asarray(inputs["chunk_embeddings"], dtype=np.float32)[0]
    mask = np.asarray(inputs["chunk_mask"])
    wkv = np.asarray(inputs["W_kv"], dtype=np.float32)
    wout = np.asarray(inputs["W_out"], dtype=np.float32)
    gamma = np.asarray(inputs["ln_gamma"], dtype=np.float32)
    beta = np.asarray(inputs["ln_beta"], dtype=np.float32)

    wk = wkv[:D]
    wvo = wout @ wkv[D:]                          # fold W_out into Wv
    wT = np.concatenate([wk, wvo], axis=0) * WPRE  # (512, L)
    wT8 = np.ascontiguousarray(q8(wT.T))           # (L, 512)
    xT8_full = np.ascontiguousarray(q8(x.T))       # (L, S)

    flat = np.ascontiguousarray(jq.reshape(J * Q, D))
    mask_on = mask != 0
    # per-key exp biases, laid out [p, b*4+st] for key = b*512+st*128+p
    dve_b = np.empty((128, 32), dtype=np.float32)
    act_b = np.empty((128, 32), dtype=np.float32)
    for col in range(32):
        b_, st = col // 4, col % 4
        keys = b_ * 512 + st * 128 + np.arange(128)
        on = mask_on[keys]
        dve_b[:, col] = np.where(on, BITS_B, -1e9)
        act_b[:, col] = np.where(on, -SHIFT, -1e30)

    in_maps = []
    for c in range(N_CORES):
        qrows = flat[c * QR:(c + 1) * QR]          # (1024, 256)
        qT = np.ascontiguousarray(qrows.T)         # (256, 1024)
        in_maps.append({
            "xT8": np.ascontiguousarray(
                xT8_full[:, c * S_LOC:(c + 1) * S_LOC]),
            "wT8": wT8,
            "qT8": np.ascontiguousarray(q8(qT)),
            "qres": np.ascontiguousarray(qrows),
            "dve_b": dve_b,
            "act_b": act_b,
            "gamma": gamma,
            "beta": beta,
        })
    return in_maps


def kernel(**inputs) -> np.ndarray:
    global _NC_CACHE
    in_maps = _make_in_maps(inputs)
    if _NC_CACHE is None:
        _NC_CACHE = build_program()
    res = run_bass_kernel_spmd(_NC_CACHE, in_maps, list(range(N_CORES)))
    out = np.concatenate([res.results[c]["y"] for c in range(N_CORES)], axis=0)
    return np.ascontiguousarray(out.reshape(J, Q, D).astype(np.float32))


# revision 15
# speedup vs baseline: 1.2537x; 1.0235x over previous
"""ChunkCrossAttention Trainium2 kernel (v2: fp8 + AllGather-KV).

Math (per reference):
  x = chunk_embeddings[0]                      # (S, L)
  k, v = split(x @ W_kv.T)                     # (S, D) each
  scores = einsum('jqd,sd->jqs', q, k) / sqrt(D), masked
  attn = softmax(scores, -1)
  out = (attn @ v) @ W_out.T + q  -> LayerNorm(gamma, beta)

Strategy (8 NeuronCores):
  - Queries sharded: each core owns 1024 q rows end-to-end (no partial
    softmax, no ReduceScatter of 8MB partials like v1).
  - W_out folded into the value projection on the host (Wvo = W_out @ Wv),
    so phase 1 emits k^T [d, s] and v' [s, d] directly in the layouts the
    attention matmuls need.
  - All matmuls fp8(e4m3) DoubleRow: 2x bf16 PE throughput. Weights are
    prescaled x64 on host (e4m3 min-normal 2^-6 vs W ~ N(0, 1/64^2));
    the psum->fp8 copies divide back by 64.
  - KV projection sharded over S (512 keys/core), then the tiny fp8 KV
    blob (264KB/core) is AllGather'd in 4 key-quarter chunks that
    pipeline behind phase-1/2 compute.
  - Softmax without max subtraction, with a global shift exp(x-3)
    (softmax-invariant) to keep fp8 exponents in range. exp runs on
    THREE engines in parallel: Act (native Exp -> fp8), DVE and GpSimd
    (Schraudolph: construct the e4m3 bit pattern directly as
    round(score*8*SCALE*log2e + const) with a saturating uint8 convert;
    negatives clamp to 0x00=+0.0, masked keys get bias -1e9 -> 0).
  - Denominator via a ones column appended to v' (DP=260 wide attn out).
  - LN scale invariance: LN(num/den + q) == LN(num + den*q), so the
    epilogue needs no reciprocal and no division.
  - Attention accumulates in PSUM across all 4096 keys: 6 q-tiles
    chunk-major (6 banks) + 2 q-tiles replayed at the end from staged
    exp tiles (frees 2 banks for the score matmuls during the main loop).
"""
import sys

sys.path.insert(0, "/opt/trn_rl_repo")

import numpy as np

import concourse.bacc as bacc
import concourse.mybir as mybir
import concourse.tile as tile
from concourse.bass_utils import run_bass_kernel_spmd

N_CORES = 8
J, Q, D = 64, 128, 256
S, L = 4096, 4096
S_LOC = S // N_CORES          # 512 keys per core
QR = (J // N_CORES) * Q       # 1024 query rows per core
DP = D + 4                    # attn free: 256 outputs + denom + 3 pad
LN_EPS = 1e-5
SCALE = 1.0 / np.sqrt(D)
SHIFT = 3.0                   # global exp shift (softmax invariant)
LOG2E = 1.4426950408889634
A_CONST = 8.0 * SCALE * LOG2E
BITS_B = 8.0 * (7.0 - SHIFT * LOG2E) - 0.5
WPRE = 64.0                   # host weight prescale before fp8 cast

F32 = mybir.dt.float32
F16 = mybir.dt.float16
FP8 = mybir.dt.float8e4
U8 = mybir.dt.uint8
AF = mybir.ActivationFunctionType
ALU = mybir.AluOpType
PM = mybir.MatmulPerfMode

N_LB = L // 256               # 16 L-pairs (256 contraction rows each)


def build_program():
    nc = bacc.Bacc(None, num_devices=N_CORES)

    xT8 = nc.declare_dram_parameter("xT8", [L, S_LOC], FP8, isOutput=False)
    wT8 = nc.declare_dram_parameter("wT8", [L, 2 * D], FP8, isOutput=False)
    qT8 = nc.declare_dram_parameter("qT8", [2 * 128, QR], FP8, isOutput=False)
    qres = nc.declare_dram_parameter("qres", [QR, D], F16, isOutput=False)
    dve_b = nc.declare_dram_parameter("dve_b", [128, 32], F32, isOutput=False)
    act_b = nc.declare_dram_parameter("act_b", [128, 32], F32, isOutput=False)
    gamma = nc.declare_dram_parameter("gamma", [D], F32, isOutput=False)
    beta = nc.declare_dram_parameter("beta", [D], F32, isOutput=False)
    y = nc.declare_dram_parameter("y", [QR, D], F16, isOutput=True)

    ag_in = [nc.dram_tensor(f"ag_in{h}", [128, 2, 516], FP8) for h in range(2)]
    ag_out = [nc.dram_tensor(f"ag_out{h}", [N_CORES, 128, 2, 516], FP8,
                             addr_space="Shared") for h in range(2)]

    import concourse.bass as bass

    with tile.TileContext(nc) as tc:
        with tc.tile_pool(name="singles", bufs=1) as singles, \
             tc.tile_pool(name="wpool", bufs=1) as wpool, \
             tc.tile_pool(name="xpool", bufs=N_LB + 4) as xpool, \
             tc.tile_pool(name="ex0", bufs=5) as ex0p, \
             tc.tile_pool(name="ex1", bufs=1) as ex1p, \
             tc.tile_pool(name="hp", bufs=6) as hpool, \
             tc.tile_pool(name="small", bufs=24) as small:

            # ---- persistent loads (scalar DMA queue; sync/vector carry
            # the phase-1 streams) ----
            qT_sb = singles.tile([128, 2, QR], FP8)
            nc.gpsimd.dma_start(out=qT_sb,
                                in_=qT8.rearrange("(c p) q -> p c q", p=128))
            dve_b_sb = singles.tile([128, 32], F32)
            nc.gpsimd.dma_start(out=dve_b_sb, in_=dve_b[:, :])
            act_b_sb = singles.tile([128, 32], F32)
            nc.gpsimd.dma_start(out=act_b_sb, in_=act_b[:, :])
            qres_sb = singles.tile([128, QR // 128, D], F16)
            for qq in range(2):
                nc.gpsimd.dma_start(
                    out=qres_sb[:, qq * 4:(qq + 1) * 4, :],
                    in_=qres.rearrange("(t p) d -> p t d", p=128)[
                        :, qq * 4:(qq + 1) * 4, :])
            g_ap = gamma[:]
            gamma_sb = singles.tile([128, D], F32)
            nc.gpsimd.dma_start(out=gamma_sb, in_=bass.AP(
                tensor=g_ap.tensor, offset=g_ap.offset,
                ap=[[0, 128], g_ap.ap[0]]))
            b_ap = beta[:]
            beta_sb = singles.tile([128, D], F32)
            nc.gpsimd.dma_start(out=beta_sb, in_=bass.AP(
                tensor=b_ap.tensor, offset=b_ap.offset,
                ap=[[0, 128], b_ap.ap[0]]))
            eps_sb = singles.tile([128, 1], F32)
            nc.vector.memset(eps_sb, LN_EPS)

            kT_loc = singles.tile([128, 2, S_LOC], FP8)
            vp_loc = singles.tile([128, 4, DP], FP8)
            nc.vector.memset(vp_loc[:, :, D:D + 1], 1.0)
            nc.vector.memset(vp_loc[:, :, D + 1:DP], 0.0)
            kT_all = singles.tile([128, N_CORES, 2, S_LOC], FP8)
            vp_all = singles.tile([128, N_CORES, 4, DP], FP8)

            # ---- phase 1: local KV projection (fp8 DR), AG per quarter ----
            ps1 = tc.tile_pool(name="ps1", bufs=1, space="PSUM")
            ps_1 = ps1.__enter__()
            kacc = [ps_1.tile([128, 2, 256], F32, name=f"kacc{h}")
                    for h in range(2)]
            vacc = [ps_1.tile([128, 256], F32, name=f"vacc{qd}")
                    for qd in range(4)]

            wts = []
            for lb in range(N_LB):
                wt = wpool.tile([128, 2, 2 * D], FP8, tag=f"wt{lb}",
                                name=f"wt{lb}")
                nc.sync.dma_start(
                    out=wt,
                    in_=wT8[lb * 256:(lb + 1) * 256, :].rearrange(
                        "(a p) n -> p a n", p=128))
                wts.append(wt)

            def kick_ag(h):
                # blob [128, 2, 516]: [:, dc, 0:256]=kT dc-chunk,
                # [:, j, 256:516]=vp quarter 2h+j
                for dc in range(2):
                    nc.gpsimd.dma_start(
                        out=ag_in[h][:, dc, 0:256],
                        in_=kT_loc[:, dc, h * 256:(h + 1) * 256])
                for j in range(2):
                    nc.gpsimd.dma_start(out=ag_in[h][:, j, 256:516],
                                        in_=vp_loc[:, 2 * h + j, :])
                nc.gpsimd.collective_compute(
                    "AllGather", ALU.bypass,
                    replica_groups=[list(range(N_CORES))],
                    ins=[ag_in[h][:, :, :]], outs=[ag_out[h][:, :, :, :]])
                gathered = ag_out[h].rearrange("c p g f -> p c g f")
                for cb in range(2):
                    cs = slice(cb * 4, (cb + 1) * 4)
                    q0 = nc.gpsimd if cb == 0 else nc.sync
                    for dc in range(2):
                        q0.dma_start(
                            out=kT_all[:, cs, dc, h * 256:(h + 1) * 256],
                            in_=gathered[:, cs, dc, 0:256])
                    for j in range(2):
                        q0.dma_start(out=vp_all[:, cs, 2 * h + j, :],
                                     in_=gathered[:, cs, j, 256:516])

            for h in range(2):
                xts = []
                for xc in range(N_LB // 2):
                    xt = xpool.tile([128, 2, 2, 256], FP8, tag="xt")
                    nc.scalar.dma_start(
                        out=xt,
                        in_=xT8[xc * 512:(xc + 1) * 512,
                                h * 256:(h + 1) * 256].rearrange(
                                    "(l a p) k -> p l a k", p=128, a=2))
                    xts.append(xt)
                for lb in range(N_LB):
                    xt_s = xts[lb // 2][:, lb % 2, :, :]
                    for kd in range(2):
                        nc.tensor.matmul(
                            kacc[h][:, kd, :],
                            wts[lb][:, :, kd * 128:(kd + 1) * 128],
                            xt_s,
                            start=(lb == 0), stop=(lb == N_LB - 1),
                            perf_mode=PM.DoubleRow)
                nc.scalar.activation(out=kT_loc[:, :, h * 256:(h + 1) * 256],
                                     in_=kacc[h], func=AF.Copy,
                                     scale=1.0 / WPRE)
                for qq in range(2):
                    qd = 2 * h + qq
                    for lb in range(N_LB):
                        nc.tensor.matmul(
                            vacc[qd],
                            xts[lb // 2][:, lb % 2, :,
                                         qq * 128:(qq + 1) * 128],
                            wts[lb][:, :, D:2 * D],
                            start=(lb == 0), stop=(lb == N_LB - 1),
                            perf_mode=PM.DoubleRow)
                    nc.scalar.activation(out=vp_loc[:, qd, 0:D], in_=vacc[qd],
                                         func=AF.Copy, scale=1.0 / WPRE)
                kick_ag(h)
            ps1.__exit__(None, None, None)

            # ---- phase 2: scores -> exp -> attention over all keys ----
            ps2a = tc.tile_pool(name="ps_at", bufs=1, space="PSUM")
            ps_at = ps2a.__enter__()
            ps2s = tc.tile_pool(name="ps_sc", bufs=2, space="PSUM")
            ps_sc = ps2s.__enter__()

            at = [ps_at.tile([128, DP], F32, name=f"at{qt}") for qt in range(6)]
            ex1_tiles = {}
            for b in range(N_CORES):
                for pc in range(2):
                    ex1_tiles[(b, pc)] = ex1p.tile(
                        [128, 2, 512], FP8, name=f"ex1_{b}_{pc}")

            # exp engine split: alternate Act / DVE per key-subtile so
            # every q row averages both engines' quantization error.
            # (gpsimd cannot read PSUM, so it only gets SBUF-side work.)
            def do_exp(ext_slice, sc, b, st, qh):
                key_idx = 4 * b + st
                pc = st // 2
                use_act = ((b + st + qh) % 2 == 0 if pc == 0
                           else (2 * b + st + qh) % 5 < 3)
                if use_act:
                    nc.scalar.activation(out=ext_slice, in_=sc, func=AF.Exp,
                                         bias=act_b_sb[:, key_idx:key_idx + 1],
                                         scale=SCALE)
                else:
                    nc.vector.tensor_scalar(out=ext_slice.bitcast(U8), in0=sc,
                                            scalar1=A_CONST,
                                            scalar2=dve_b_sb[:, key_idx:key_idx + 1],
                                            op0=ALU.mult, op1=ALU.add)

            for pc in range(2):
                for b in range(N_CORES):
                    ex0t = ex0p.tile([128, 2, 512], FP8, tag="ex0",
                                     name="ex0t")
                    ex_pair = [ex0t, ex1_tiles[(b, pc)]]
                    for qh in range(2):
                        for sub in range(2):
                            st = 2 * pc + sub
                            sc = ps_sc.tile([128, 512], F32, tag="sc")
                            nc.tensor.matmul(
                                sc,
                                kT_all[:, b, :, st * 128:(st + 1) * 128],
                                qT_sb[:, :, qh * 512:(qh + 1) * 512],
                                start=True, stop=True,
                                perf_mode=PM.DoubleRow)
                            do_exp(ex_pair[qh][:, sub, :], sc, b, st, qh)
                    for qt in range(6):
                        qh, qi = (0, qt) if qt < 4 else (1, qt - 4)
                        nc.tensor.matmul(
                            at[qt], ex_pair[qh][:, :, qi * 128:(qi + 1) * 128],
                            vp_all[:, b, 2 * pc:2 * pc + 2, :],
                            start=(pc == 0 and b == 0),
                            stop=(pc == 1 and b == N_CORES - 1),
                            perf_mode=PM.DoubleRow)

            ps2s.__exit__(None, None, None)

            # deferred q-tiles 6,7 replay staged exp against vp
            ps2d = tc.tile_pool(name="ps_d", bufs=1, space="PSUM")
            ps_d = ps2d.__enter__()
            at_d = [ps_d.tile([128, DP], F32, name=f"atd{i}") for i in range(2)]
            for i, qt in enumerate((6, 7)):
                first = True
                for pc in range(2):
                    for b in range(N_CORES):
                        nc.tensor.matmul(
                            at_d[i],
                            ex1_tiles[(b, pc)][:, :, (qt - 4) * 128:(qt - 3) * 128],
                            vp_all[:, b, 2 * pc:2 * pc + 2, :],
                            start=first, stop=(pc == 1 and b == N_CORES - 1),
                            perf_mode=PM.DoubleRow)
                        first = False

            # ---- epilogue: h' = num + den*q, LN (scale-invariant) ----
            y_r = y.rearrange("(t p) d -> t p d", p=128)

            def epilogue(qt, acc):
                den = small.tile([128, 1], F32, tag="den")
                nc.vector.tensor_copy(out=den, in_=acc[:, D:D + 1])
                h = hpool.tile([128, D], F32, tag="h")
                nc.vector.tensor_scalar_mul(out=h, in0=qres_sb[:, qt, :],
                                            scalar1=den)
                nc.vector.tensor_add(out=h, in0=h, in1=acc[:, 0:D])
                stats = small.tile([128, 6], F32, tag="stats")
                nc.vector.bn_stats(out=stats, in_=h)
                mv = small.tile([128, 2], F32, tag="mv")
                nc.vector.bn_aggr(out=mv, in_=stats)
                rstd = small.tile([128, 1], F32, tag="rstd")
                nc.scalar.activation(out=rstd, in_=mv[:, 1:2], func=AF.Sqrt,
                                     bias=eps_sb, scale=1.0)
                nc.vector.reciprocal(out=rstd, in_=rstd)
                nmr = small.tile([128, 1], F32, tag="nmr")
                nc.vector.tensor_scalar(out=nmr, in0=mv[:, 0:1],
                                        scalar1=rstd, scalar2=-1.0,
                                        op0=ALU.mult, op1=ALU.mult)
                xh = hpool.tile([128, D], F32, tag="xh")
                nc.scalar.activation(out=xh, in_=h, func=AF.Identity,
                                     bias=nmr, scale=rstd)
                yt = hpool.tile([128, D], F16, tag="yt")
                nc.gpsimd.tensor_mul(out=yt, in0=xh, in1=gamma_sb)
                nc.gpsimd.tensor_add(out=yt, in0=yt, in1=beta_sb)
                nc.sync.dma_start(out=y_r[qt], in_=yt)

            for qt in range(6):
                epilogue(qt, at[qt])
            for i, qt in enumerate((6, 7)):
                epilogue(qt, at_d[i])

            ps2d.__exit__(None, None, None)
            ps2a.__exit__(None, None, None)

    nc.finalize()
    return nc


_NC_CACHE = None


def _make_in_maps(inputs):
    import ml_dtypes
    e4 = ml_dtypes.float8_e4m3

    def q8(a):
        return np.clip(a, -240.0, 240.0).astype(e4)

    jq = np.asarray(inputs["justice_queries"], dtype=np.float32)
    x = np.asarray(inputs["chunk_embeddings"], dtype=np.float32)[0]
    mask = np.asarray(inputs["chunk_mask"])
    wkv = np.asarray(inputs["W_kv"], dtype=np.float32)
    wout = np.asarray(inputs["W_out"], dtype=np.float32)
    gamma = np.asarray(inputs["ln_gamma"], dtype=np.float32)
    beta = np.asarray(inputs["ln_beta"], dtype=np.float32)

    wk = wkv[:D]
    wvo = wout @ wkv[D:]                          # fold W_out into Wv
    wT = np.concatenate([wk, wvo], axis=0) * WPRE  # (512, L)
    wT8 = np.ascontiguousarray(q8(wT.T))           # (L, 512)
    xT8_full = np.ascontiguousarray(q8(x.T))       # (L, S)

    flat = np.ascontiguousarray(jq.reshape(J * Q, D))
    mask_on = mask != 0
    # per-key exp biases, laid out [p, b*4+st] for key = b*512+st*128+p
    dve_b = np.empty((128, 32), dtype=np.float32)
    act_b = np.empty((128, 32), dtype=np.float32)
    for col in range(32):
        b_, st = col // 4, col % 4
        keys = b_ * 512 + st * 128 + np.arange(128)
        on = mask_on[keys]
        dve_b[:, col] = np.where(on, BITS_B, -1e9)
        act_b[:, col] = np.where(on, -SHIFT, -1e30)

    in_maps = []
    for c in range(N_CORES):
        qrows = flat[c * QR:(c + 1) * QR]          # (1024, 256)
        qT = np.ascontiguousarray(qrows.T)         # (256, 1024)
        in_maps.append({
            "xT8": np.ascontiguousarray(
                xT8_full[:, c * S_LOC:(c + 1) * S_LOC]),
            "wT8": wT8,
            "qT8": np.ascontiguousarray(q8(qT)),
            "qres": np.ascontiguousarray(qrows.astype(np.float16)),
            "dve_b": dve_b,
            "act_b": act_b,
            "gamma": gamma,
            "beta": beta,
        })
    return in_maps


def kernel(**inputs) -> np.ndarray:
    global _NC_CACHE
    in_maps = _make_in_maps(inputs)
    if _NC_CACHE is None:
        _NC_CACHE = build_program()
    res = run_bass_kernel_spmd(_NC_CACHE, in_maps, list(range(N_CORES)))
    out = np.concatenate([res.results[c]["y"] for c in range(N_CORES)], axis=0)
    return np.ascontiguousarray(out.reshape(J, Q, D).astype(np.float32))
